# revision 34
# baseline (speedup 1.0000x reference)
"""Trainium2 Bass kernel for CoarseMatching (dual-softmax + mutual-argmax).

Math (per batch n):
    sim  = (f3d @ fq.T) / (C * (TEMP+1e-4))                 [L, S]
    E    = exp(sim)            (unstable-but-bounded softmax numerator)
    conf = E^2 / (rowsum[l] * colsum[s])                    [L, S]
    rowmax/rowarg over s, colmax over l -> mutual-NN match extraction
    (threshold > 0.2, border h>=2 & w>=2 on the 60x80 grid)

Sharding: 8 cores; core c handles batch n = c//2, row-half h = c%2
(1024 of the 2048 db rows). The axis-1 softmax denominator (colsum,
a sum over L) is completed with an AllReduce over each pair of cores.

Device outputs per core: conf block [1024,4800], per-row max/argmax,
per-column max (partition-packed). Host finishes the tiny [N,L]-sized
match extraction exactly as the reference does.
"""

import numpy as np

import concourse.bass as bass
import concourse.mybir as mybir
import concourse.tile as tile
from concourse import bacc
from concourse.bass import ts
from concourse.bass_utils import run_bass_kernel_spmd
from concourse.masks import make_identity

F32 = mybir.dt.float32
F32R = mybir.dt.float32r
U32 = mybir.dt.uint32

# Problem constants (hardcoded per spec)
N, L, S, C = 4, 2048, 4800, 256
H1C, W1C = 60, 80
TEMPERATURE = 0.08
THR = 0.2
BORDER_RM = 2
SCALE_IMG = 480.0 / 60.0  # 8.0
N_CORES = 8
HALVES = 2
M = L // HALVES           # rows per core
KT = C // 128             # K subtiles

# Tiling
A_CHUNK = 480             # stage-A moving chunk
A_SUB = 480               # matmul moving sub-chunk (<=512, one PSUM bank each)
B_CHUNK = 960             # stage-B chunk
CS_SUB = 480              # colsum partition-reduce matmul chunk (<=512)
PSUM_BANK = 512           # f32 elements per PSUM bank

REPLICA_GROUPS = [[2 * i, 2 * i + 1] for i in range(N_CORES // 2)]


def build_nc(M=M, S=S, C=C, a_chunk=A_CHUNK, a_sub=A_SUB, b_chunk=B_CHUNK,
             cs_sub=CS_SUB, use_mask=False, replica_groups=REPLICA_GROUPS,
             variant="full"):
    """Build the SPMD Bass program (identical on all cores)."""
    KTl = C // 128
    T = M // 128
    a_nch = S // a_chunk
    b_nch = S // b_chunk
    nsub = a_chunk // a_sub
    nc128 = (S + 127) // 128
    scale_sim = float((1.0 / np.sqrt(np.float32(C), dtype=np.float32) ** 2)
                      / np.float32(np.float32(TEMPERATURE) + np.float32(1e-4)))

    nc = bacc.Bacc(None, target_bir_lowering=False)
    f3dT_d = nc.declare_dram_parameter("f3dT", [C, M], F32, isOutput=False)
    fqT_d = nc.declare_dram_parameter("fqT", [C, S], F32, isOutput=False)
    if use_mask:
        maskf_d = nc.declare_dram_parameter("maskf", [1, S], F32, isOutput=False)
    conf_d = nc.declare_dram_parameter("conf", [M, S], F32, isOutput=True)
    rowmax_d = nc.declare_dram_parameter("rowmax", [128, T], F32, isOutput=True)
    rowarg_d = nc.declare_dram_parameter("rowarg", [128, T], U32, isOutput=True)
    colmax_d = nc.declare_dram_parameter("colmax", [128, nc128], F32, isOutput=True)

    f3dT_v = f3dT_d[:, :].rearrange("(k p) m -> p k m", p=128)
    fqT_v = fqT_d[:, :].rearrange("(k p) s -> p k s", p=128)

    with tile.TileContext(nc) as tc:
        with (
            tc.tile_pool(name="persist", bufs=1) as pp,
            tc.tile_pool(name="mov", bufs=2) as mv,
            tc.tile_pool(name="psum_mm", bufs=4, space="PSUM") as pmm,
            tc.tile_pool(name="psum_cs", bufs=2, space="PSUM") as pcs,
            tc.tile_pool(name="psum_tp", bufs=2, space="PSUM") as ptp,
            tc.tile_pool(name="dram", bufs=1, space="DRAM") as dp,
        ):
            # Persistent SBUF
            E_sb = [pp.tile([128, S], F32, tag=f"E{t}", name=f"E{t}")
                    for t in range(T)]
            # f3d (stage A) shares its slot with colmax_acc (stage B)
            f3d_sb = pp.tile([128, KTl, M], F32, tag="f3c")
            # colsum accumulate (stage A) then replicated 1/colsum (stage B)
            big_scratch = pp.tile([128, S], F32, tag="bigs")
            rs_part = pp.tile([128, T * a_nch], F32, tag="rs_part")
            rm_part = pp.tile([128, T * b_nch], F32, tag="rm_part")
            rowsum_sb = pp.tile([128, T], F32, tag="rowsum")
            inv_rs = pp.tile([128, T], F32, tag="inv_rs")
            sqrt_inv_rs = pp.tile([128, T], F32, tag="sqrt_inv_rs")
            rowmax_sb = pp.tile([128, T], F32, tag="rowmax")
            rowarg_sb = pp.tile([128, T], U32, tag="rowarg")
            maxq = pp.tile([128, 8], F32, tag="maxq")
            rowarg8 = pp.tile([128, 8], U32, tag="rowarg8")
            colmax_sb = pp.tile([128, nc128], F32, tag="colmax_sb")
            ones_sb = pp.tile([128, 1], F32, tag="ones")
            ones_row = pp.tile([1, 128], F32, tag="ones_row")
            ident = pp.tile([128, 128], F32, tag="ident")

            colsum_loc = dp.tile([1, S], F32, tag="cs_loc")
            colsum_glob = dp.tile([1, S], F32, tag="cs_glob")

            nc.vector.memset(ones_sb[:, :], 1.0)
            nc.vector.memset(ones_row[:, :], 1.0)
            if S % 128 != 0:
                nc.vector.memset(colmax_sb[:, :], 0.0)
            make_identity(nc, ident[:, :])

            # Load f3dT fully (resident)
            nc.gpsimd.dma_start(out=f3d_sb[:, :, :], in_=f3dT_v)

            colsum_acc = big_scratch

            # ---------------- Stage A: sim -> E, rowsum, colsum ----------
            for c in range(a_nch):
                csl = slice(c * a_chunk, (c + 1) * a_chunk)
                fq_sb = mv.tile([128, KTl, a_chunk], F32, tag="mv_fq",
                                bufs=3)
                nc.gpsimd.dma_start(out=fq_sb[:, :, :], in_=fqT_v[:, :, csl])
                if use_mask:
                    mk_sb = mv.tile([128, a_chunk], F32, tag="mv_mk")
                    nc.sync.dma_start(
                        out=mk_sb[:, :],
                        in_=maskf_d[0:1, csl].to_broadcast([128, a_chunk]),
                    )
                for t in range(T):
                    # each a_sub-wide matmul gets its own PSUM bank slot
                    ps = pmm.tile([128, nsub * PSUM_BANK], F32, tag="ps_mm")
                    for j in range(nsub):
                        jsl = slice(j * a_sub, (j + 1) * a_sub)
                        psl = slice(j * PSUM_BANK, j * PSUM_BANK + a_sub)
                        for k in range(KTl):
                            nc.tensor.matmul(
                                ps[:, psl],
                                lhsT=f3d_sb[:, k, ts(t, 128)],
                                rhs=fq_sb[:, k, jsl],
                                start=(k == 0),
                                stop=(k == KTl - 1),
                            )
                    ps_view = ps[:, :].rearrange(
                        "p (j b) -> p j b", b=PSUM_BANK)[:, :, 0:a_sub]
                    e_view = E_sb[t][:, csl].rearrange(
                        "p (j x) -> p j x", x=a_sub)
                    nc.scalar.activation(
                        out=e_view,
                        in_=ps_view,
                        func=mybir.ActivationFunctionType.Exp,
                        scale=scale_sim,
                        accum_out=rs_part[:, t * a_nch + c: t * a_nch + c + 1],
                    )
                    if use_mask:
                        nc.vector.tensor_mul(E_sb[t][:, csl], E_sb[t][:, csl],
                                             mk_sb[:, :])
                    if t == 0:
                        nc.vector.tensor_copy(colsum_acc[:, csl],
                                              E_sb[t][:, csl])
                    else:
                        nc.vector.tensor_add(colsum_acc[:, csl],
                                             colsum_acc[:, csl],
                                             E_sb[t][:, csl])

            # rowsum -> sqrt(1/rowsum)
            if use_mask:
                # accum_out summed pre-mask exp; recompute rowsum from E
                for t in range(T):
                    nc.vector.reduce_sum(
                        out=rowsum_sb[:, t:t + 1], in_=E_sb[t][:, :],
                        axis=mybir.AxisListType.X)
            else:
                for t in range(T):
                    nc.vector.reduce_sum(
                        out=rowsum_sb[:, t:t + 1],
                        in_=rs_part[:, t * a_nch:(t + 1) * a_nch],
                        axis=mybir.AxisListType.X)
            nc.vector.reciprocal(out=inv_rs[:, :], in_=rowsum_sb[:, :])
            nc.scalar.sqrt(out=sqrt_inv_rs[:, :], in_=inv_rs[:, :])

            # colsum partition-reduce via ones-matmul, ship to DRAM
            for q in range(S // cs_sub):
                qsl = slice(q * cs_sub, (q + 1) * cs_sub)
                cps = pcs.tile([1, cs_sub], F32, tag="ps_cs")
                nc.tensor.matmul(
                    cps[:, :],
                    lhsT=ones_sb[:, :],
                    rhs=colsum_acc[:, qsl],
                    start=True, stop=True,
                )
                cs_stage = pp.tile([1, cs_sub], F32, tag="mv_cs",
                                   name="cs_stage", bufs=2)
                nc.vector.tensor_copy(cs_stage[:, :], cps[:, :])
                nc.sync.dma_start(out=colsum_loc[0:1, qsl], in_=cs_stage[:, :])

            if variant == "A":
                pass
            elif variant == "nocc":
                nc.sync.dma_start(out=colsum_glob[0:1, :],
                                  in_=colsum_loc[0:1, :])
            else:
                nc.gpsimd.collective_compute(
                    "AllReduce",
                    mybir.AluOpType.add,
                    replica_groups=replica_groups,
                    ins=[colsum_loc[0:1, :]],
                    outs=[colsum_glob[0:1, :]],
                )

            # replicated 1/colsum (reuses colsum_acc buffer)
            cs_rep = big_scratch
            if variant == "A":
                pass
            elif variant == "nobcast":
                # replicate via PE (K=1 ones matmul) instead of stride-0 DMA
                csr_sb = pp.tile([1, S], F32, tag="csr_sb", name="csr_sb")
                nc.sync.dma_start(out=csr_sb[0:1, :], in_=colsum_glob[0:1, :])
                for q in range(S // cs_sub):
                    qsl = slice(q * cs_sub, (q + 1) * cs_sub)
                    rps = pcs.tile([128, cs_sub], F32, tag="ps_rep")
                    nc.tensor.matmul(rps[:, :], lhsT=ones_row[:, :],
                                     rhs=csr_sb[0:1, qsl], start=True,
                                     stop=True)
                    nc.vector.tensor_copy(cs_rep[:, qsl], rps[:, :])
            else:
                nc.sync.dma_start(out=cs_rep[:, :],
                                  in_=colsum_glob[0:1, :].to_broadcast([128, S]))
            if use_mask:
                nc.vector.tensor_scalar_max(cs_rep[:, :], cs_rep[:, :], 1e-35)
            if variant != "A":
                nc.vector.reciprocal(out=cs_rep[:, :], in_=cs_rep[:, :])

            # ---------------- Stage B: conf, row/col stats ---------------
            # reuses the f3d slot (stage A is done with it)
            colmax_acc = pp.tile([128, S], F32, tag="f3c", name="colmax_acc")
            for t in (range(T) if variant != "A" else []):
                for c in range(b_nch):
                    csl = slice(c * b_chunk, (c + 1) * b_chunk)
                    # same byte size as the fq chunk slot -> share it
                    G = mv.tile([128, b_chunk], F32, tag="mv_fq", bufs=3)
                    nc.scalar.activation(
                        out=G[:, :],
                        in_=E_sb[t][:, csl],
                        func=mybir.ActivationFunctionType.Square,
                        scale=sqrt_inv_rs[:, t:t + 1],
                    )
                    if variant == "nottr":
                        nc.vector.tensor_mul(E_sb[t][:, csl], G[:, :],
                                             cs_rep[:, csl])
                    else:
                        nc.vector.tensor_tensor_reduce(
                            out=E_sb[t][:, csl],
                            in0=G[:, :],
                            in1=cs_rep[:, csl],
                            scale=1.0,
                            scalar=-3.0e38,
                            op0=mybir.AluOpType.mult,
                            op1=mybir.AluOpType.max,
                            accum_out=rm_part[:,
                                              t * b_nch + c: t * b_nch + c + 1],
                        )
                    if t == 0:
                        nc.vector.tensor_copy(colmax_acc[:, csl],
                                              E_sb[t][:, csl])
                    else:
                        nc.vector.tensor_max(
                            out=colmax_acc[:, csl], in0=colmax_acc[:, csl],
                            in1=E_sb[t][:, csl])
                # row stats for this tile
                if variant == "nottr":
                    nc.vector.reduce_max(out=rowmax_sb[:, t:t + 1],
                                         in_=E_sb[t][:, :],
                                         axis=mybir.AxisListType.X)
                else:
                    nc.vector.reduce_max(
                        out=rowmax_sb[:, t:t + 1],
                        in_=rm_part[:, t * b_nch:(t + 1) * b_nch],
                        axis=mybir.AxisListType.X)
                nc.vector.tensor_copy(maxq[:, 0:1], rowmax_sb[:, t:t + 1])
                nc.vector.tensor_copy(maxq[:, 1:2], maxq[:, 0:1])
                nc.vector.tensor_copy(maxq[:, 2:4], maxq[:, 0:2])
                nc.vector.tensor_copy(maxq[:, 4:8], maxq[:, 0:4])
                nc.vector.max_index(out=rowarg8[:, :], in_max=maxq[:, :],
                                    in_values=E_sb[t][:, :])
                nc.vector.tensor_copy(rowarg_sb[:, t:t + 1], rowarg8[:, 0:1])
                nc.sync.dma_start(out=conf_d[ts(t, 128), :], in_=E_sb[t][:, :])

            # colmax partition-reduce via PE transpose
            for q in (range(nc128) if variant != "A" else []):
                cw = min(128, S - q * 128)
                tp = ptp.tile([cw, 128], F32, tag="ps_tp")
                nc.tensor.transpose(
                    out=tp[:, :],
                    in_=colmax_acc[:, q * 128: q * 128 + cw],
                    identity=ident[:, :],
                )
                nc.vector.reduce_max(out=colmax_sb[0:cw, q:q + 1],
                                     in_=tp[:, :],
                                     axis=mybir.AxisListType.X)

            if variant != "A":
                nc.sync.dma_start(out=rowmax_d[:, :], in_=rowmax_sb[:, :])
                nc.sync.dma_start(out=rowarg_d[:, :], in_=rowarg_sb[:, :])
                nc.sync.dma_start(out=colmax_d[:, :], in_=colmax_sb[:, :])
            else:
                nc.sync.dma_start(out=conf_d[0:128, :], in_=E_sb[0][:, :])

    nc.compile()
    return nc


_NC_CACHE = {}
DEFAULT_VARIANT = "nottr"


def _get_nc(use_mask):
    if use_mask not in _NC_CACHE:
        _NC_CACHE[use_mask] = build_nc(use_mask=use_mask,
                                       variant=DEFAULT_VARIANT)
    return _NC_CACHE[use_mask]


def _run_device(in_maps, use_mask, trace=False):
    nc = _get_nc(use_mask)
    return run_bass_kernel_spmd(nc, in_maps, list(range(N_CORES)), trace=trace)


def kernel(feat_db_3d, feat_query, keypoints3d, mask_query, _trace=False,
           _results_out=None):
    feat_db_3d = np.ascontiguousarray(np.asarray(feat_db_3d, dtype=np.float32))
    feat_query = np.asarray(feat_query, dtype=np.float32)
    keypoints3d = np.asarray(keypoints3d, dtype=np.float32)
    mask_query = np.asarray(mask_query)

    mask_bool = mask_query.astype(bool)
    # all-True and all-False both reduce to the unmasked math (the -1e9
    # column bias cancels inside each softmax when applied uniformly)
    per_n_any = mask_bool.any(axis=1)
    per_n_all = mask_bool.all(axis=1)
    use_mask = bool(np.any(per_n_any & ~per_n_all))

    fqT = [np.ascontiguousarray(feat_query[n].T) for n in range(N)]
    in_maps = []
    for c in range(N_CORES):
        n, h = c // HALVES, c % HALVES
        m = {
            "f3dT": np.ascontiguousarray(
                feat_db_3d[n, h * M:(h + 1) * M, :].T),
            "fqT": fqT[n],
        }
        if use_mask:
            mf = np.where(per_n_any[n] & ~per_n_all[n],
                          mask_bool[n].astype(np.float32),
                          np.ones(S, np.float32))
            m["maskf"] = np.ascontiguousarray(mf.reshape(1, S))
        in_maps.append(m)

    res = _run_device(in_maps, use_mask, trace=_trace)
    if _results_out is not None:
        _results_out.append(res)
    outs = res.results

    T = M // 128
    nc128 = (S + 127) // 128
    conf = np.empty((N, L, S), dtype=np.float32)
    rowmax = np.empty((N, L), dtype=np.float32)
    rowarg = np.empty((N, L), dtype=np.int64)
    colmax = np.empty((N, S), dtype=np.float32)
    for n in range(N):
        parts = []
        for h in range(HALVES):
            o = outs[n * HALVES + h]
            conf[n, h * M:(h + 1) * M, :] = o["conf"].reshape(M, S)
            rowmax[n, h * M:(h + 1) * M] = o["rowmax"].reshape(128, T).T.ravel()
            rowarg[n, h * M:(h + 1) * M] = (
                o["rowarg"].reshape(128, T).T.ravel().astype(np.int64))
            parts.append(o["colmax"].reshape(128, nc128).T.ravel()[:S])
        colmax[n] = np.maximum(parts[0], parts[1])

    # match extraction (tiny [N, L] work, identical to the reference)
    border_ok = (rowarg // W1C >= BORDER_RM) & (rowarg % W1C >= BORDER_RM)
    col_at_arg = np.take_along_axis(colmax, rowarg, axis=1)
    mask_v = (rowmax > THR) & border_ok & (rowmax >= col_at_arg)
    all_j_ids = np.where(mask_v, rowarg, 0).astype(np.int32)
    mconf = np.where(mask_v, rowmax, np.float32(0.0)).astype(np.float32)
    mkpts_query = (np.stack([all_j_ids % W1C, all_j_ids // W1C], axis=-1)
                   .astype(np.float32) * np.float32(SCALE_IMG))
    return (conf, mask_v, all_j_ids, mconf, mkpts_query, keypoints3d)


# revision 37
# speedup vs baseline: 1.0787x; 1.0787x over previous
"""Trainium2 Bass kernel for CoarseMatching (dual-softmax + mutual-argmax).

Math (per batch n):
    sim  = (f3d @ fq.T) / (C * (TEMP+1e-4))                 [L, S]
    E    = exp(sim)            (unstable-but-bounded softmax numerator)
    conf = E^2 / (rowsum[l] * colsum[s])                    [L, S]
    rowmax/rowarg over s, colmax over l -> mutual-NN match extraction
    (threshold > 0.2, border h>=2 & w>=2 on the 60x80 grid)

Sharding: 8 cores; core c handles batch n = c//2, row-half h = c%2
(1024 of the 2048 db rows). The axis-1 softmax denominator (colsum,
a sum over L) is completed with an AllReduce over each pair of cores.

Device outputs per core: conf block [1024,4800], per-row max/argmax,
per-column max (partition-packed). Host finishes the tiny [N,L]-sized
match extraction exactly as the reference does.
"""

import numpy as np

import concourse.bass as bass
import concourse.mybir as mybir
import concourse.tile as tile
from concourse import bacc
from concourse.bass import ts
from concourse.bass_utils import run_bass_kernel_spmd
from concourse.masks import make_identity

F32 = mybir.dt.float32
BF16 = mybir.dt.bfloat16
F32R = mybir.dt.float32r
U32 = mybir.dt.uint32

# Problem constants (hardcoded per spec)
N, L, S, C = 4, 2048, 4800, 256
H1C, W1C = 60, 80
TEMPERATURE = 0.08
THR = 0.2
BORDER_RM = 2
SCALE_IMG = 480.0 / 60.0  # 8.0
N_CORES = 8
HALVES = 2
M = L // HALVES           # rows per core
KT = C // 128             # K subtiles

# Tiling
A_CHUNK = 480             # stage-A moving chunk
A_SUB = 480               # matmul moving sub-chunk (<=512, one PSUM bank each)
B_CHUNK = 960             # stage-B chunk
CS_SUB = 480              # colsum partition-reduce matmul chunk (<=512)
PSUM_BANK = 512           # f32 elements per PSUM bank

REPLICA_GROUPS = [[2 * i, 2 * i + 1] for i in range(N_CORES // 2)]


def build_nc(M=M, S=S, C=C, a_chunk=A_CHUNK, a_sub=A_SUB, b_chunk=B_CHUNK,
             cs_sub=CS_SUB, use_mask=False, replica_groups=REPLICA_GROUPS,
             variant="full", bf16split=False):
    """Build the SPMD Bass program (identical on all cores)."""
    KTl = C // 128
    T = M // 128
    a_nch = S // a_chunk
    b_nch = S // b_chunk
    nsub = a_chunk // a_sub
    nc128 = (S + 127) // 128
    scale_sim = float((1.0 / np.sqrt(np.float32(C), dtype=np.float32) ** 2)
                      / np.float32(np.float32(TEMPERATURE) + np.float32(1e-4)))

    nc = bacc.Bacc(None, target_bir_lowering=False)
    mm_dt = BF16 if bf16split else F32
    nparts = 2 if bf16split else 1  # hi/lo input pairs
    f3dT_d = nc.declare_dram_parameter("f3dT", [nparts, C, M], mm_dt,
                                       isOutput=False)
    fqT_d = nc.declare_dram_parameter("fqT", [nparts, C, S], mm_dt,
                                      isOutput=False)
    if use_mask:
        maskf_d = nc.declare_dram_parameter("maskf", [1, S], F32, isOutput=False)
    conf_d = nc.declare_dram_parameter("conf", [M, S], F32, isOutput=True)
    rowmax_d = nc.declare_dram_parameter("rowmax", [128, T], F32, isOutput=True)
    rowarg_d = nc.declare_dram_parameter("rowarg", [128, T], U32, isOutput=True)
    colmax_d = nc.declare_dram_parameter("colmax", [128, nc128], F32, isOutput=True)

    f3dT_v = f3dT_d[:, :, :].rearrange("h (k p) m -> p h k m", p=128)
    fqT_v = fqT_d[:, :, :].rearrange("h (k p) s -> p h k s", p=128)

    with tile.TileContext(nc) as tc:
        with (
            tc.tile_pool(name="persist", bufs=1) as pp,
            tc.tile_pool(name="mov", bufs=2) as mv,
            tc.tile_pool(name="psum_mm", bufs=2, space="PSUM") as pmm,
            tc.tile_pool(name="psum_cs", bufs=2, space="PSUM") as pcs,
            tc.tile_pool(name="psum_tp", bufs=2, space="PSUM") as ptp,
            tc.tile_pool(name="dram", bufs=1, space="DRAM") as dp,
        ):
            # Persistent SBUF
            E_sb = [pp.tile([128, S], F32, tag=f"E{t}", name=f"E{t}")
                    for t in range(T)]
            # f3d (stage A) shares its slot with colmax_acc (stage B)
            f3d_sb = pp.tile([128, nparts, KTl, M], mm_dt, tag="f3c")
            # colsum accumulate (stage A) then replicated 1/colsum (stage B)
            big_scratch = pp.tile([128, S], F32, tag="bigs")
            rs_part = pp.tile([128, T * a_nch], F32, tag="rs_part")
            rm_part = pp.tile([128, T * b_nch], F32, tag="rm_part")
            rowsum_sb = pp.tile([128, T], F32, tag="rowsum")
            inv_rs = pp.tile([128, T], F32, tag="inv_rs")
            sqrt_inv_rs = pp.tile([128, T], F32, tag="sqrt_inv_rs")
            rowmax_sb = pp.tile([128, T], F32, tag="rowmax")
            rowarg_sb = pp.tile([128, T], U32, tag="rowarg")
            maxq = pp.tile([128, 8], F32, tag="maxq")
            rowarg8 = pp.tile([128, 8], U32, tag="rowarg8")
            colmax_sb = pp.tile([128, nc128], F32, tag="colmax_sb")
            ones_sb = pp.tile([128, 1], F32, tag="ones")
            ones_row = pp.tile([1, 128], F32, tag="ones_row")
            ident = pp.tile([128, 128], F32, tag="ident")

            colsum_loc = dp.tile([1, S], F32, tag="cs_loc")
            colsum_glob = dp.tile([1, S], F32, tag="cs_glob")

            nc.vector.memset(ones_sb[:, :], 1.0)
            nc.vector.memset(ones_row[:, :], 1.0)
            if S % 128 != 0:
                nc.vector.memset(colmax_sb[:, :], 0.0)
            make_identity(nc, ident[:, :])

            # Load f3dT fully (resident)
            nc.gpsimd.dma_start(out=f3d_sb[:, :, :, :], in_=f3dT_v)

            colsum_acc = big_scratch

            # ---------------- Stage A: sim -> E, rowsum, colsum ----------
            for c in range(a_nch):
                csl = slice(c * a_chunk, (c + 1) * a_chunk)
                fq_sb = mv.tile([128, nparts, KTl, a_chunk], mm_dt,
                                tag="mv_fq")
                nc.gpsimd.dma_start(out=fq_sb[:, :, :, :],
                                    in_=fqT_v[:, :, :, csl])
                if use_mask:
                    mk_sb = mv.tile([128, a_chunk], F32, tag="mv_mk")
                    nc.sync.dma_start(
                        out=mk_sb[:, :],
                        in_=maskf_d[0:1, csl].to_broadcast([128, a_chunk]),
                    )
                for t in range(T):
                    # each a_sub-wide matmul gets its own PSUM bank slot
                    ps = pmm.tile([128, nsub * PSUM_BANK], F32, tag="ps_mm")
                    for j in range(nsub):
                        jsl = slice(j * a_sub, (j + 1) * a_sub)
                        psl = slice(j * PSUM_BANK, j * PSUM_BANK + a_sub)
                        if bf16split:
                            # x = hi+lo; sim ~ hi.hi + hi.lo + lo.hi
                            combos = [(0, 0), (0, 1), (1, 0)]
                            for ci, (ha, hb) in enumerate(combos):
                                for k in range(KTl):
                                    nc.tensor.matmul(
                                        ps[:, psl],
                                        lhsT=f3d_sb[:, ha, k, ts(t, 128)],
                                        rhs=fq_sb[:, hb, k, jsl],
                                        start=(ci == 0 and k == 0),
                                        stop=(ci == 2 and k == KTl - 1),
                                    )
                        else:
                            for k in range(KTl):
                                nc.tensor.matmul(
                                    ps[:, psl],
                                    lhsT=f3d_sb[:, 0, k, ts(t, 128)],
                                    rhs=fq_sb[:, 0, k, jsl],
                                    start=(k == 0),
                                    stop=(k == KTl - 1),
                                )
                    ps_view = ps[:, :].rearrange(
                        "p (j b) -> p j b", b=PSUM_BANK)[:, :, 0:a_sub]
                    e_view = E_sb[t][:, csl].rearrange(
                        "p (j x) -> p j x", x=a_sub)
                    nc.scalar.activation(
                        out=e_view,
                        in_=ps_view,
                        func=mybir.ActivationFunctionType.Exp,
                        scale=scale_sim,
                        accum_out=rs_part[:, t * a_nch + c: t * a_nch + c + 1],
                    )
                    if use_mask:
                        nc.vector.tensor_mul(E_sb[t][:, csl], E_sb[t][:, csl],
                                             mk_sb[:, :])
                    if t == 0:
                        nc.vector.tensor_copy(colsum_acc[:, csl],
                                              E_sb[t][:, csl])
                    else:
                        nc.vector.tensor_add(colsum_acc[:, csl],
                                             colsum_acc[:, csl],
                                             E_sb[t][:, csl])

            # rowsum -> sqrt(1/rowsum)
            if use_mask:
                # accum_out summed pre-mask exp; recompute rowsum from E
                for t in range(T):
                    nc.vector.reduce_sum(
                        out=rowsum_sb[:, t:t + 1], in_=E_sb[t][:, :],
                        axis=mybir.AxisListType.X)
            else:
                for t in range(T):
                    nc.vector.reduce_sum(
                        out=rowsum_sb[:, t:t + 1],
                        in_=rs_part[:, t * a_nch:(t + 1) * a_nch],
                        axis=mybir.AxisListType.X)
            nc.vector.reciprocal(out=inv_rs[:, :], in_=rowsum_sb[:, :])
            nc.scalar.sqrt(out=sqrt_inv_rs[:, :], in_=inv_rs[:, :])

            # colsum partition-reduce via ones-matmul, ship to DRAM
            for q in range(S // cs_sub):
                qsl = slice(q * cs_sub, (q + 1) * cs_sub)
                cps = pcs.tile([1, cs_sub], F32, tag="ps_cs")
                nc.tensor.matmul(
                    cps[:, :],
                    lhsT=ones_sb[:, :],
                    rhs=colsum_acc[:, qsl],
                    start=True, stop=True,
                )
                cs_stage = pp.tile([1, cs_sub], F32, tag="mv_cs",
                                   name="cs_stage", bufs=2)
                nc.vector.tensor_copy(cs_stage[:, :], cps[:, :])
                nc.sync.dma_start(out=colsum_loc[0:1, qsl], in_=cs_stage[:, :])

            if variant == "A":
                pass
            elif variant == "nocc":
                nc.sync.dma_start(out=colsum_glob[0:1, :],
                                  in_=colsum_loc[0:1, :])
            else:
                nc.gpsimd.collective_compute(
                    "AllReduce",
                    mybir.AluOpType.add,
                    replica_groups=replica_groups,
                    ins=[colsum_loc[0:1, :]],
                    outs=[colsum_glob[0:1, :]],
                )

            # replicated 1/colsum (reuses colsum_acc buffer)
            cs_rep = big_scratch
            if variant == "A":
                pass
            elif variant == "nobcast":
                # replicate via PE (K=1 ones matmul) instead of stride-0 DMA
                csr_sb = pp.tile([1, S], F32, tag="csr_sb", name="csr_sb")
                nc.sync.dma_start(out=csr_sb[0:1, :], in_=colsum_glob[0:1, :])
                for q in range(S // cs_sub):
                    qsl = slice(q * cs_sub, (q + 1) * cs_sub)
                    rps = pcs.tile([128, cs_sub], F32, tag="ps_rep")
                    nc.tensor.matmul(rps[:, :], lhsT=ones_row[:, :],
                                     rhs=csr_sb[0:1, qsl], start=True,
                                     stop=True)
                    nc.vector.tensor_copy(cs_rep[:, qsl], rps[:, :])
            else:
                nc.sync.dma_start(out=cs_rep[:, :],
                                  in_=colsum_glob[0:1, :].to_broadcast([128, S]))
            if use_mask:
                nc.vector.tensor_scalar_max(cs_rep[:, :], cs_rep[:, :], 1e-35)
            if variant != "A":
                nc.vector.reciprocal(out=cs_rep[:, :], in_=cs_rep[:, :])

            # ---------------- Stage B: conf, row/col stats ---------------
            # reuses the f3d slot (stage A is done with it)
            colmax_acc = pp.tile([128, S], F32, tag="f3c", name="colmax_acc")
            for t in (range(T) if variant != "A" else []):
                for c in range(b_nch):
                    csl = slice(c * b_chunk, (c + 1) * b_chunk)
                    # same byte size as the fq chunk slot -> share it
                    G = mv.tile([128, b_chunk], F32, tag="mv_fq")
                    nc.scalar.activation(
                        out=G[:, :],
                        in_=E_sb[t][:, csl],
                        func=mybir.ActivationFunctionType.Square,
                        scale=sqrt_inv_rs[:, t:t + 1],
                    )
                    if variant == "nottr":
                        nc.vector.tensor_mul(E_sb[t][:, csl], G[:, :],
                                             cs_rep[:, csl])
                    else:
                        nc.vector.tensor_tensor_reduce(
                            out=E_sb[t][:, csl],
                            in0=G[:, :],
                            in1=cs_rep[:, csl],
                            scale=1.0,
                            scalar=-3.0e38,
                            op0=mybir.AluOpType.mult,
                            op1=mybir.AluOpType.max,
                            accum_out=rm_part[:,
                                              t * b_nch + c: t * b_nch + c + 1],
                        )
                    if t == 0:
                        nc.vector.tensor_copy(colmax_acc[:, csl],
                                              E_sb[t][:, csl])
                    else:
                        nc.vector.tensor_max(
                            out=colmax_acc[:, csl], in0=colmax_acc[:, csl],
                            in1=E_sb[t][:, csl])
                # row stats for this tile
                if variant == "nottr":
                    nc.vector.reduce_max(out=rowmax_sb[:, t:t + 1],
                                         in_=E_sb[t][:, :],
                                         axis=mybir.AxisListType.X)
                else:
                    nc.vector.reduce_max(
                        out=rowmax_sb[:, t:t + 1],
                        in_=rm_part[:, t * b_nch:(t + 1) * b_nch],
                        axis=mybir.AxisListType.X)
                nc.vector.tensor_copy(maxq[:, 0:1], rowmax_sb[:, t:t + 1])
                nc.vector.tensor_copy(maxq[:, 1:2], maxq[:, 0:1])
                nc.vector.tensor_copy(maxq[:, 2:4], maxq[:, 0:2])
                nc.vector.tensor_copy(maxq[:, 4:8], maxq[:, 0:4])
                nc.vector.max_index(out=rowarg8[:, :], in_max=maxq[:, :],
                                    in_values=E_sb[t][:, :])
                nc.vector.tensor_copy(rowarg_sb[:, t:t + 1], rowarg8[:, 0:1])
                nc.sync.dma_start(out=conf_d[ts(t, 128), :], in_=E_sb[t][:, :])

            # colmax partition-reduce via PE transpose
            for q in (range(nc128) if variant != "A" else []):
                cw = min(128, S - q * 128)
                tp = ptp.tile([cw, 128], F32, tag="ps_tp")
                nc.tensor.transpose(
                    out=tp[:, :],
                    in_=colmax_acc[:, q * 128: q * 128 + cw],
                    identity=ident[:, :],
                )
                nc.vector.reduce_max(out=colmax_sb[0:cw, q:q + 1],
                                     in_=tp[:, :],
                                     axis=mybir.AxisListType.X)

            if variant != "A":
                nc.sync.dma_start(out=rowmax_d[:, :], in_=rowmax_sb[:, :])
                nc.sync.dma_start(out=rowarg_d[:, :], in_=rowarg_sb[:, :])
                nc.sync.dma_start(out=colmax_d[:, :], in_=colmax_sb[:, :])
            else:
                nc.sync.dma_start(out=conf_d[0:128, :], in_=E_sb[0][:, :])

    nc.compile()
    return nc


_NC_CACHE = {}
DEFAULT_VARIANT = "nottr"
USE_BF16_SPLIT = True


def _get_nc(use_mask):
    if use_mask not in _NC_CACHE:
        _NC_CACHE[use_mask] = build_nc(use_mask=use_mask,
                                       variant=DEFAULT_VARIANT,
                                       bf16split=USE_BF16_SPLIT)
    return _NC_CACHE[use_mask]


def _mm_input(x):
    """Pack a [C, X] f32 operand for the device matmul: either [1, C, X]
    f32, or hi/lo bf16 split [2, C, X] (x ~ hi + lo to ~2^-17 rel)."""
    x = np.ascontiguousarray(x, dtype=np.float32)
    if not USE_BF16_SPLIT:
        return x[None]
    import ml_dtypes
    hi = x.astype(ml_dtypes.bfloat16)
    lo = (x - hi.astype(np.float32)).astype(ml_dtypes.bfloat16)
    return np.ascontiguousarray(np.stack([hi, lo], axis=0))


def _run_device(in_maps, use_mask, trace=False):
    nc = _get_nc(use_mask)
    return run_bass_kernel_spmd(nc, in_maps, list(range(N_CORES)), trace=trace)


def kernel(feat_db_3d, feat_query, keypoints3d, mask_query, _trace=False,
           _results_out=None):
    feat_db_3d = np.ascontiguousarray(np.asarray(feat_db_3d, dtype=np.float32))
    feat_query = np.asarray(feat_query, dtype=np.float32)
    keypoints3d = np.asarray(keypoints3d, dtype=np.float32)
    mask_query = np.asarray(mask_query)

    mask_bool = mask_query.astype(bool)
    # all-True and all-False both reduce to the unmasked math (the -1e9
    # column bias cancels inside each softmax when applied uniformly)
    per_n_any = mask_bool.any(axis=1)
    per_n_all = mask_bool.all(axis=1)
    use_mask = bool(np.any(per_n_any & ~per_n_all))

    fqT = [_mm_input(feat_query[n].T) for n in range(N)]
    in_maps = []
    for c in range(N_CORES):
        n, h = c // HALVES, c % HALVES
        m = {
            "f3dT": _mm_input(feat_db_3d[n, h * M:(h + 1) * M, :].T),
            "fqT": fqT[n],
        }
        if use_mask:
            mf = np.where(per_n_any[n] & ~per_n_all[n],
                          mask_bool[n].astype(np.float32),
                          np.ones(S, np.float32))
            m["maskf"] = np.ascontiguousarray(mf.reshape(1, S))
        in_maps.append(m)

    res = _run_device(in_maps, use_mask, trace=_trace)
    if _results_out is not None:
        _results_out.append(res)
    outs = res.results

    T = M // 128
    nc128 = (S + 127) // 128
    conf = np.empty((N, L, S), dtype=np.float32)
    rowmax = np.empty((N, L), dtype=np.float32)
    rowarg = np.empty((N, L), dtype=np.int64)
    colmax = np.empty((N, S), dtype=np.float32)
    for n in range(N):
        parts = []
        for h in range(HALVES):
            o = outs[n * HALVES + h]
            conf[n, h * M:(h + 1) * M, :] = o["conf"].reshape(M, S)
            rowmax[n, h * M:(h + 1) * M] = o["rowmax"].reshape(128, T).T.ravel()
            rowarg[n, h * M:(h + 1) * M] = (
                o["rowarg"].reshape(128, T).T.ravel().astype(np.int64))
            parts.append(o["colmax"].reshape(128, nc128).T.ravel()[:S])
        colmax[n] = np.maximum(parts[0], parts[1])

    # match extraction (tiny [N, L] work, identical to the reference)
    border_ok = (rowarg // W1C >= BORDER_RM) & (rowarg % W1C >= BORDER_RM)
    col_at_arg = np.take_along_axis(colmax, rowarg, axis=1)
    mask_v = (rowmax > THR) & border_ok & (rowmax >= col_at_arg)
    all_j_ids = np.where(mask_v, rowarg, 0).astype(np.int32)
    mconf = np.where(mask_v, rowmax, np.float32(0.0)).astype(np.float32)
    mkpts_query = (np.stack([all_j_ids % W1C, all_j_ids // W1C], axis=-1)
                   .astype(np.float32) * np.float32(SCALE_IMG))
    return (conf, mask_v, all_j_ids, mconf, mkpts_query, keypoints3d)
